# revision 23
# baseline (speedup 1.0000x reference)
"""Trainium2 Bass kernel for a 3-layer GraphConv GNN (PyG GraphConv, aggr='add').

Reference math (per layer):  x' = x @ Wr + segment_sum(x[src], dst) @ Wn + b
Final head:                  y  = clip(x3 @ Wout + bout, -4, 4)

Distribution: nodes are partitioned across 8 NeuronCores (dst/graph parallel).
Each core aggregates the in-edges of its node shard with banked int16
`dma_gather` (the packed hi/lo-bf16 x table lives in HBM), reduces the
gathered edge rows into per-128-node windows with one-hot matmuls on the
tensor engine (S built on-chip by the vector engine from an iota/is_equal),
applies the two GEMMs per chunk, and AllGathers the new packed x between
layers.

Numerics: x is carried as bf16 hi + bf16 lo (error-compensated split,
~1e-5 relative); the segment sum accumulates in fp32 PSUM; GEMMs are fp32.

The host side relabels nodes (pads each shard of 12500 to 12544 rows),
sorts edges by (core, chunk, bank), pads per-(chunk,bank) segments to the
max count over cores so that all 8 cores run an identical SPMD program, and
precomputes the int16 gather indices plus the per-tile dst-local columns
("dstloc") the S-builder compares against.
"""

import math
import os
import sys
from dataclasses import dataclass, field

import numpy as np

sys.path.insert(0, "/opt/trn_rl_repo")

P = 128  # partitions


@dataclass(frozen=True)
class Cfg:
    n_real: int = 100000
    e: int = 1200000
    d: int = 64
    nc: int = 8
    banks: int = 4
    stage_chunks: int = 8
    n_layers: int = 3

    @property
    def shard(self) -> int:
        # pad per-core shard to a multiple of 128
        return math.ceil(math.ceil(self.n_real / self.nc) / P) * P

    @property
    def shard_real(self) -> int:
        return math.ceil(self.n_real / self.nc)

    @property
    def npad(self) -> int:
        return self.shard * self.nc

    @property
    def quarter(self) -> int:
        # per-core shard quarter; quarter q of every core concatenates into
        # gather bank q (one Shared AllGather per quarter per layer)
        assert self.shard % self.banks == 0
        return self.shard // self.banks

    @property
    def bank_rows(self) -> int:
        assert self.npad % self.banks == 0
        return self.npad // self.banks

    @property
    def chunks(self) -> int:
        return self.shard // P

    @property
    def dd(self) -> int:
        return 2 * self.d  # packed hi|lo row width


FULL = Cfg()


# ---------------------------------------------------------------------------
# Host-side graph preprocessing
# ---------------------------------------------------------------------------

@dataclass
class Schedule:
    """Core-independent program layout (identical for all 8 SPMD cores)."""
    cfg: Cfg
    stages: list  # list of list of chunk ids
    # per (stage, bank): number of gather tiles and idx-buffer column offset
    nt: dict = field(default_factory=dict)       # (s,b) -> n_tiles
    idx_col0: dict = field(default_factory=dict)  # (s,b) -> col offset in idx buffer
    idx_cols_total: int = 0
    # instances: per stage, ordered list of (bank, tile, chunk_local, scol)
    inst: dict = field(default_factory=dict)      # s -> list
    n_scols: int = 0
    # per chunk: list of (bank, tile, scol) in emission order
    chunk_inst: dict = field(default_factory=dict)  # k -> list
    # stage-major full-x layout: row offset of stage s block (8 cores wide)
    stage_row0: list = field(default_factory=list)
    # per (stage, bank): unrounded max-over-cores index count
    num_idxs: dict = field(default_factory=dict)


def _relabel(cfg: Cfg, arr: np.ndarray) -> np.ndarray:
    """old node id -> padded (core, shard-local) id: c*shard + p."""
    return (arr // cfg.shard_real) * cfg.shard + (arr % cfg.shard_real)


def _stage_layout(cfg: Cfg, stages):
    """Row offsets of per-stage blocks in the stage-major full-x layout.

    Full layout: for each stage s: 8 contiguous per-core slices of
    R_s = len(stage)*128 rows each. AllGather of a stage's shard slice
    lands exactly in its block.
    """
    row0 = []
    off = 0
    for st in stages:
        row0.append(off)
        off += cfg.nc * len(st) * P
    assert off == cfg.npad
    return row0


def _pos_of_shard_id(cfg: Cfg, sched: Schedule, shard_ids: np.ndarray):
    """(c*shard + p) -> row in the quarter-major full-x layout.

    Bank q of the table holds quarter q of every core's shard:
    pos = q*(nc*quarter) + c*quarter + (p % quarter). Each bank is then
    exactly the output of one AllGather over the cores' quarter-q slices.
    """
    c = shard_ids // cfg.shard
    p = shard_ids % cfg.shard
    q = p // cfg.quarter
    return q * (cfg.nc * cfg.quarter) + c * cfg.quarter + (p % cfg.quarter)


def build_schedule_and_data(cfg: Cfg, edge_index: np.ndarray):
    """Returns (sched, per_core_idx_i16[PxC], per_core_dstloc[PxJ]) arrays."""
    src_sh = _relabel(cfg, edge_index[0].astype(np.int64))
    dst_sh = _relabel(cfg, edge_index[1].astype(np.int64))

    stages = [
        list(range(s, min(s + cfg.stage_chunks, cfg.chunks)))
        for s in range(0, cfg.chunks, cfg.stage_chunks)
    ]
    sched = Schedule(cfg=cfg, stages=stages)
    sched.stage_row0 = _stage_layout(cfg, stages)

    src_pos = _pos_of_shard_id(cfg, sched, src_sh)  # row in stage-major layout
    core = dst_sh // cfg.shard
    dst_p = dst_sh % cfg.shard          # shard-local dst position
    chunk = dst_p // P
    dstage = chunk // cfg.stage_chunks
    bank = src_pos // cfg.bank_rows

    # per-core edges sorted by (dst stage, src bank, dst chunk)
    order = {}
    for c in range(cfg.nc):
        m = np.nonzero(core == c)[0]
        key = (dstage[m] * cfg.banks + bank[m]) * cfg.chunks + chunk[m]
        order[c] = m[np.argsort(key, kind="stable")]

    # per (core, stage, bank) segment boundaries + per-(chunk) sub-boundaries
    nsb = len(stages) * cfg.banks
    seg_bounds = {}   # (c) -> searchsorted boundaries over (s,b)
    sub_bounds = {}   # (c) -> boundaries over (s,b,chunk)
    for c in range(cfg.nc):
        o = order[c]
        sb_key = dstage[o] * cfg.banks + bank[o]
        seg_bounds[c] = np.searchsorted(sb_key, np.arange(nsb + 1))
        sub_key = sb_key * cfg.chunks + chunk[o]
        sub_bounds[c] = np.searchsorted(
            sub_key, np.arange(nsb * cfg.chunks + 1))

    # --- core-independent layout: L_sb = max over cores, %128 ---
    scol = 0
    idx_col = 0
    for si, st in enumerate(stages):
        raw = []
        for b in range(cfg.banks):
            sb = si * cfg.banks + b
            seg_len = max(
                int(seg_bounds[c][sb + 1] - seg_bounds[c][sb])
                for c in range(cfg.nc)
            )
            seg_len = ((max(seg_len, 16) + 15) // 16) * 16
            n_tiles = max(1, math.ceil(seg_len / P))
            sched.num_idxs[(si, b)] = seg_len
            sched.nt[(si, b)] = n_tiles
            sched.idx_col0[(si, b)] = idx_col
            idx_col += n_tiles * P // 16
            # candidate windows per tile: union over cores of overlapping
            # chunks in that core's cumulative layout
            cand = [set() for _ in range(n_tiles)]
            for c in range(cfg.nc):
                base = seg_bounds[c][sb]
                cums = sub_bounds[c][
                    sb * cfg.chunks:sb * cfg.chunks + cfg.chunks + 1
                ] - base
                for kl, k in enumerate(st):
                    i0, i1 = int(cums[k]), int(cums[k + 1])
                    if i1 <= i0:
                        continue
                    for t in range(i0 // P, (i1 - 1) // P + 1):
                        if t < n_tiles:
                            cand[t].add(kl)
            for t in range(n_tiles):
                for kl in sorted(cand[t]):
                    raw.append((b, t, kl))
        # chunk-major instance order so S-tile groups are built in the same
        # order the per-chunk matmuls consume them (avoids slot deadlock)
        raw.sort(key=lambda r: (r[2], r[0], r[1]))
        sched.inst[si] = []
        for (b, t, kl) in raw:
            sched.inst[si].append((b, t, kl, scol))
            sched.chunk_inst.setdefault(st[kl], []).append((b, t, scol))
            scol += 1
    sched.n_scols = scol
    sched.idx_cols_total = idx_col
    for k in range(cfg.chunks):
        assert sched.chunk_inst.get(k), f"chunk {k} has no instances"

    # --- per-core data arrays ---
    idx_bufs = []
    dstloc_bufs = []
    cnt_bufs = []
    for c in range(cfg.nc):
        o = order[c]
        csrc = src_pos[o]
        cdstp = dst_p[o]
        idx_flat = np.zeros(sched.idx_cols_total * 16, dtype=np.int16)
        dstloc = np.full((P, sched.n_scols), -512.0, dtype=np.float32)
        cnt = np.zeros(len(stages) * cfg.banks, dtype=np.int32)

        for si, st in enumerate(stages):
            for b in range(cfg.banks):
                sb = si * cfg.banks + b
                i0 = int(seg_bounds[c][sb])
                i1 = int(seg_bounds[c][sb + 1])
                n = i1 - i0
                n_tiles = sched.nt[(si, b)]
                L = n_tiles * P
                # per-core count rounded to the 16-lane wrap granularity
                n16 = min(max(((n + 15) // 16) * 16, 16), L)
                cnt[sb] = n16
                pos_idx = np.zeros(L, dtype=np.int64)
                pos_dst = np.full(L, -(1 << 30), dtype=np.int64)
                pos_idx[:n] = csrc[i0:i1] - b * cfg.bank_rows
                pos_dst[:n] = cdstp[i0:i1]
                assert n == 0 or (
                    pos_idx[:n].min() >= 0 and pos_idx[:n].max() < cfg.bank_rows)
                col0 = sched.idx_col0[(si, b)]
                idx_flat[col0 * 16:col0 * 16 + L] = pos_idx.astype(np.int16)
                for (bb, t, kl, j) in sched.inst[si]:
                    if bb != b:
                        continue
                    k = st[kl]
                    seg = pos_dst[t * P:(t + 1) * P]
                    col = (seg - k * P).astype(np.float64)
                    col[(col < 0) | (col >= P)] = -512.0
                    dstloc[:, j] = col.astype(np.float32)

        wrapped16 = idx_flat.reshape(-1, 16).T  # [16, cols]
        idx_bufs.append(np.ascontiguousarray(np.tile(wrapped16, (P // 16, 1))))
        dstloc_bufs.append(dstloc)
        cnt_bufs.append(cnt)

    return sched, idx_bufs, dstloc_bufs, cnt_bufs


def pack_hi_lo(x: np.ndarray) -> np.ndarray:
    """[n, d] f32 -> [n, 2d] bf16 packed (hi | lo)."""
    import ml_dtypes
    hi = x.astype(ml_dtypes.bfloat16)
    lo = (x - hi.astype(np.float32)).astype(ml_dtypes.bfloat16)
    return np.concatenate([hi, lo], axis=1)


# ---------------------------------------------------------------------------
# Bass program
# ---------------------------------------------------------------------------

def build_program(cfg: Cfg, sched: Schedule):
    from concourse import bacc, bass, mybir, tile
    from concourse.library_config import mlp

    f32 = mybir.dt.float32
    bf16 = mybir.dt.bfloat16
    i16 = mybir.dt.int16

    nc = bacc.Bacc("TRN2", target_bir_lowering=False, num_swdge_queues=4)

    # --- parameters ---
    xp = nc.dram_tensor("xp", [cfg.npad, cfg.dd], bf16, kind="ExternalInput")
    xs = nc.dram_tensor("xs", [cfg.shard, cfg.dd], bf16, kind="ExternalInput")
    idxp = nc.dram_tensor("idx", [P, sched.idx_cols_total], i16, kind="ExternalInput")
    dlp = nc.dram_tensor("dstloc", [P, sched.n_scols], bf16, kind="ExternalInput")
    n_sb = len(sched.stages) * cfg.banks
    gcntp = nc.dram_tensor("gcnt", [1, n_sb], mybir.dt.int32, kind="ExternalInput")
    iotap = nc.dram_tensor("iota", [P, P], bf16, kind="ExternalInput")
    id16p = nc.dram_tensor("id16", [P, P], bf16, kind="ExternalInput")
    idfp = nc.dram_tensor("idf", [P, P], f32, kind="ExternalInput")
    wrp = [nc.dram_tensor(f"wr{l}", [cfg.d, cfg.d], f32, kind="ExternalInput") for l in range(3)]
    wnp = [nc.dram_tensor(f"wn{l}", [cfg.d, cfg.d], f32, kind="ExternalInput") for l in range(3)]
    bp = [nc.dram_tensor(f"bias{l}", [P, cfg.d], f32, kind="ExternalInput") for l in range(3)]
    woutp = nc.dram_tensor("woutb", [P, cfg.d], f32, kind="ExternalInput")
    boutp = nc.dram_tensor("boutb", [P, 1], f32, kind="ExternalInput")
    outp = nc.dram_tensor("out", [cfg.shard, 1], f32, kind="ExternalOutput")

    max_nt = max(sched.nt.values())
    SGRP = 16  # S-build instances per vector op
    # Cap on indices per dma_gather instruction: large instructions overflow
    # the SWDGE descriptor ring and hang the hardware.
    GT = int(os.environ.get("GNN_GATHER_TILES", "32"))  # tiles per gather

    with tile.TileContext(nc) as tc:
        with (
            tc.tile_pool(name="res", bufs=1) as res,
            tc.tile_pool(name="gb", bufs=3) as gbp,
            tc.tile_pool(name="gself", bufs=3) as gsp,
            tc.tile_pool(name="sbuf_s", bufs=4) as ssp,
            tc.tile_pool(name="work", bufs=3) as wkp,
            tc.tile_pool(name="outt", bufs=3) as otp,
            tc.tile_pool(name="pcat", bufs=2, space="PSUM") as pcat,
            tc.tile_pool(name="ptr", bufs=2, space="PSUM") as ptr,
            tc.tile_pool(name="pxt", bufs=2, space="PSUM") as pxt,
            tc.tile_pool(name="po", bufs=2, space="PSUM") as pop,
            tc.tile_pool(name="dram", bufs=1, space="DRAM") as dram,
        ):
            nc.gpsimd.load_library(mlp)
            # resident tiles
            idx_sb = res.tile([P, sched.idx_cols_total], i16)
            nc.sync.dma_start(out=idx_sb[:], in_=idxp[:])
            dl_sb = res.tile([P, sched.n_scols], bf16)
            nc.sync.dma_start(out=dl_sb[:], in_=dlp[:])
            cnt_sb = res.tile([1, n_sb], mybir.dt.int32)
            nc.sync.dma_start(out=cnt_sb[:], in_=gcntp[:])
            gregs = [
                nc.alloc_register(mybir.EngineType.Pool, f"gcnt{b}")
                for b in range(cfg.banks)
            ]
            # Gather destination buffers: manually rotated resident tiles
            # (NOT pool slots) so a one-time memset provably zeroes every
            # buffer — unwritten tails must stay finite for the masked
            # matmuls (0 x NaN would poison PSUM).
            GROT = int(os.environ.get("GNN_GROT", "4"))
            gbufs = [
                [
                    res.tile([P, max_nt, cfg.dd], bf16, name=f"gbuf{b}_{r}",
                             tag=f"gbuf{b}_{r}")
                    for r in range(GROT)
                ]
                for b in range(cfg.banks)
            ]
            for b in range(cfg.banks):
                for r in range(GROT):
                    nc.vector.memset(gbufs[b][r][:], 0)
            iota_sb = res.tile([P, P], bf16)
            nc.sync.dma_start(out=iota_sb[:], in_=iotap[:])
            id16_sb = res.tile([P, P], bf16)
            nc.sync.dma_start(out=id16_sb[:], in_=id16p[:])
            idf_sb = res.tile([P, P], f32)
            nc.sync.dma_start(out=idf_sb[:], in_=idfp[:])
            wr_sb, wn_sb, wn_hi_sb, b_sb = [], [], [], []
            for l in range(3):
                w1 = res.tile([cfg.d, cfg.d], f32, tag=f"wr{l}")
                nc.sync.dma_start(out=w1[:], in_=wrp[l][:])
                w2 = res.tile([cfg.d, cfg.d], f32, tag=f"wn{l}")
                nc.sync.dma_start(out=w2[:], in_=wnp[l][:])
                # Wn replica parked at partitions 64:128 so the aggT lo rows
                # (base partition 64) can matmul against it
                w2b = res.tile([P, cfg.d], f32, tag=f"wnhi{l}")
                nc.sync.dma_start(out=w2b[cfg.d:P, :], in_=wnp[l][:])
                bb = res.tile([P, cfg.d], f32, tag=f"b{l}")
                nc.sync.dma_start(out=bb[:], in_=bp[l][:])
                wr_sb.append(w1)
                wn_sb.append(w2)
                wn_hi_sb.append(w2b)
                b_sb.append(bb)
            wout_sb = res.tile([P, cfg.d], f32)
            nc.sync.dma_start(out=wout_sb[:], in_=woutp[:])
            bout_sb = res.tile([P, 1], f32)
            nc.sync.dma_start(out=bout_sb[:], in_=boutp[:])

            # inter-layer DRAM buffers
            # per-layer full-shard output tile; quarter q slices feed the
            # quarter-q AllGather as soon as the covering chunks are written
            nxs = [
                dram.tile([cfg.shard, cfg.dd], bf16, name=f"nxs{l}",
                          tag=f"nxs{l}")
                for l in range(2)
            ]
            # per-quarter duplicates: whole-tensor AllGather inputs
            nxq = [
                [
                    dram.tile([cfg.quarter, cfg.dd], bf16,
                              name=f"nxq{l}_{q}", tag=f"nxq{l}_{q}")
                    for q in range(cfg.banks)
                ]
                for l in range(2)
            ]
            ag_addr_space = (
                "Shared" if os.environ.get("GNN_SHARED_AG", "1") == "1"
                else "Local"
            )
            # one Shared table tensor per gather bank (single-writer AllGather)
            nxf = [
                [
                    dram.tile([cfg.bank_rows, cfg.dd], bf16,
                              name=f"nxf{l}_{q}", tag=f"nxf{l}_{q}",
                              addr_space=ag_addr_space)
                    for q in range(cfg.banks)
                ]
                for l in range(2)
            ]
            # last chunk covering quarter q: issue AllGather q right after it
            q_last_chunk = {
                ((q + 1) * cfg.quarter - 1) // P: q for q in range(cfg.banks)
            }

            for l in range(cfg.n_layers):
                if l == 0:
                    xbanks = [
                        xp[b * cfg.bank_rows:(b + 1) * cfg.bank_rows, :]
                        for b in range(cfg.banks)
                    ]
                    xshard = [
                        xs[st[0] * P:(st[-1] + 1) * P, :]
                        for st in sched.stages
                    ]
                else:
                    xbanks = [nxf[l - 1][b][:] for b in range(cfg.banks)]
                    xshard = [
                        nxs[l - 1][st[0] * P:(st[-1] + 1) * P, :]
                        for st in sched.stages
                    ]

                for si, st in enumerate(sched.stages):
                    # ---- gathers ----
                    gtiles = []
                    for b in range(cfg.banks):
                        ntb = sched.nt[(si, b)]
                        g = gbufs[b][(l * len(sched.stages) + si) % GROT]
                        c0 = sched.idx_col0[(si, b)]
                        use_reg = GT >= ntb and os.environ.get(
                            "GNN_PERCORE_CNT", "0") == "1"
                        qn = b % 4  # spread gathers over the 4 SWDGE queues
                        if use_reg:
                            Lm = sched.num_idxs[(si, b)]
                            sb_i = si * cfg.banks + b
                            nc.gpsimd.reg_load(
                                gregs[b], cnt_sb[0:1, sb_i:sb_i + 1])
                            nc.gpsimd.dma_gather(
                                out_ap=g[:, :ntb, :],
                                in_ap=xbanks[b],
                                idxs_ap=idx_sb[:, c0:c0 + (Lm + 15) // 16],
                                num_idxs=Lm,
                                num_idxs_reg=gregs[b],
                                elem_size=cfg.dd,
                                single_packet=False,
                                queue_num=qn,
                            )
                        elif GT >= ntb:
                            # one gather, unrounded static count (row-0 pads
                            # fill to a multiple of 16; stale tail tiles are
                            # masked by S and memset-finite)
                            Lm = sched.num_idxs[(si, b)]
                            nc.gpsimd.dma_gather(
                                out_ap=g[:, :ntb, :],
                                in_ap=xbanks[b],
                                idxs_ap=idx_sb[:, c0:c0 + (Lm + 15) // 16],
                                num_idxs=Lm,
                                num_idxs_reg=Lm,
                                elem_size=cfg.dd,
                                single_packet=False,
                                queue_num=qn,
                            )
                        else:
                            for t0 in range(0, ntb, GT):
                                t1 = min(t0 + GT, ntb)
                                L = (t1 - t0) * P
                                nc.gpsimd.dma_gather(
                                    out_ap=g[:, t0:t1, :],
                                    in_ap=xbanks[b],
                                    idxs_ap=idx_sb[
                                        :, c0 + t0 * 8:c0 + t0 * 8 + L // 16],
                                    num_idxs=L,
                                    num_idxs_reg=L,
                                    elem_size=cfg.dd,
                                    single_packet=(
                                        os.environ.get(
                                            "GNN_SINGLE_PACKET", "0") == "1"
                                    ),
                                    queue_num=qn,
                                )
                        gtiles.append(g)
                    nst = len(st)
                    gs = gsp.tile([P, len(sched.stages[0]), cfg.dd], bf16, tag="gs")
                    nc.sync.dma_start(
                        out=gs[:, :nst, :],
                        in_=xshard[si].rearrange("(n p) d -> p n d", p=P),
                    )

                    # ---- S builds (vector engine, batched) ----
                    insts = sched.inst[si]
                    s_tiles = {}
                    j0 = insts[0][3]
                    for g0 in range(0, len(insts), SGRP):
                        grp = insts[g0:g0 + SGRP]
                        gsz = len(grp)
                        stile = ssp.tile([P, SGRP, P], bf16, tag="s")
                        a = grp[0][3]
                        nc.vector.tensor_tensor(
                            out=stile[:, :gsz, :],
                            in0=dl_sb[:, a:a + gsz, None].to_broadcast([P, gsz, P]),
                            in1=iota_sb[:, None, :].to_broadcast([P, gsz, P]),
                            op=mybir.AluOpType.is_equal,
                        )
                        for q, (b, t, kl, j) in enumerate(grp):
                            s_tiles[j] = (stile, q)

                    # ---- per chunk: reduce + GEMM ----
                    for kl, k in enumerate(st):
                        cinst = sched.chunk_inst[k]
                        pc = pcat.tile([P, cfg.dd], f32, space="PSUM")
                        for ii, (b, t, j) in enumerate(cinst):
                            stile, q = s_tiles[j]
                            nc.tensor.matmul(
                                out=pc[:],
                                lhsT=stile[:, q, :],
                                rhs=gtiles[b][:, t, :],
                                start=(ii == 0),
                                stop=(ii == len(cinst) - 1),
                            )
                        cat_sb = wkp.tile([P, cfg.dd], f32, tag="catsb")
                        nc.scalar.copy(cat_sb[:], pc[:])
                        pat = ptr.tile([cfg.d, P], f32, space="PSUM")
                        nc.tensor.matmul(
                            out=pat[:], lhsT=cat_sb[:, 0:cfg.d], rhs=idf_sb[:],
                            is_transpose=True, start=True, stop=False,
                        )
                        nc.tensor.matmul(
                            out=pat[:], lhsT=cat_sb[:, cfg.d:cfg.dd], rhs=idf_sb[:],
                            is_transpose=True, start=False, stop=True,
                        )
                        pxt_t = pxt.tile([cfg.d, P], f32, space="PSUM")
                        nc.tensor.matmul(
                            out=pxt_t[:],
                            lhsT=gs[:, kl, 0:cfg.d],
                            rhs=id16_sb[:],
                            start=True,
                            stop=False,
                        )
                        nc.tensor.matmul(
                            out=pxt_t[:],
                            lhsT=gs[:, kl, cfg.d:cfg.dd],
                            rhs=id16_sb[:],
                            start=False,
                            stop=True,
                        )
                        aggT = wkp.tile([cfg.d, P], f32, tag="aggT")
                        nc.scalar.copy(aggT[:], pat[:])
                        xT = wkp.tile([cfg.d, P], f32, tag="xT")
                        nc.scalar.copy(xT[:], pxt_t[:])
                        po = pop.tile([P, cfg.d], f32, space="PSUM")
                        nc.tensor.matmul(
                            out=po[:], lhsT=xT[:], rhs=wr_sb[l][:], start=True, stop=False
                        )
                        nc.tensor.matmul(
                            out=po[:], lhsT=aggT[:], rhs=wn_sb[l][:], start=False, stop=True
                        )
                        out_sb = otp.tile([P, cfg.d], f32, tag="osb")
                        nc.vector.tensor_add(out_sb[:], po[:], b_sb[l][:])
                        if l < cfg.n_layers - 1:
                            pk = otp.tile([P, cfg.dd], bf16, tag="pk")
                            nc.scalar.copy(pk[:, 0:cfg.d], out_sb[:])
                            nc.vector.tensor_tensor(
                                out=pk[:, cfg.d:cfg.dd],
                                in0=out_sb[:],
                                in1=pk[:, 0:cfg.d],
                                op=mybir.AluOpType.subtract,
                            )
                            nc.sync.dma_start(
                                out=nxs[l][k * P:(k + 1) * P, :],
                                in_=pk[:],
                            )
                            # duplicate into the per-quarter AllGather input
                            # tiles (chunks straddling a quarter boundary
                            # split into two partition-sliced writes)
                            g0, g1 = k * P, (k + 1) * P
                            for qq in range(g0 // cfg.quarter,
                                            (g1 - 1) // cfg.quarter + 1):
                                r0 = max(g0, qq * cfg.quarter)
                                r1 = min(g1, (qq + 1) * cfg.quarter)
                                nc.sync.dma_start(
                                    out=nxq[l][qq][
                                        r0 - qq * cfg.quarter:
                                        r1 - qq * cfg.quarter, :,
                                    ],
                                    in_=pk[r0 - g0:r1 - g0, :],
                                )
                            # quarter q fully written -> ship it while the
                            # remaining chunks compute
                            if k in q_last_chunk and os.environ.get(
                                    "GNN_EARLY_AG", "0") == "1":
                                qq = q_last_chunk[k]
                                nc.gpsimd.collective_compute(
                                    "AllGather",
                                    mybir.AluOpType.bypass,
                                    replica_groups=[list(range(cfg.nc))],
                                    ins=[nxq[l][qq][:]],
                                    outs=[nxf[l][qq][:]],
                                )
                        else:
                            tmp = otp.tile([P, cfg.d], f32, tag="htmp")
                            nc.vector.tensor_mul(tmp[:], out_sb[:], wout_sb[:])
                            red = otp.tile([P, 1], f32, tag="hred")
                            nc.vector.tensor_reduce(
                                out=red[:],
                                in_=tmp[:],
                                axis=mybir.AxisListType.X,
                                op=mybir.AluOpType.add,
                            )
                            red2 = otp.tile([P, 1], f32, tag="hred2")
                            nc.vector.tensor_tensor(
                                out=red2[:], in0=red[:], in1=bout_sb[:],
                                op=mybir.AluOpType.add,
                            )
                            red3 = otp.tile([P, 1], f32, tag="hred3")
                            nc.vector.tensor_scalar(
                                out=red3[:], in0=red2[:],
                                scalar1=4.0, scalar2=-4.0,
                                op0=mybir.AluOpType.min, op1=mybir.AluOpType.max,
                            )
                            nc.sync.dma_start(
                                out=outp[k * P:(k + 1) * P, :], in_=red3[:]
                            )

                if l < cfg.n_layers - 1 and os.environ.get(
                        "GNN_EARLY_AG", "0") != "1" and si == len(
                            sched.stages) - 1:
                    # layer-end issue: one AllGather per quarter/bank
                    for qq in range(cfg.banks):
                        nc.gpsimd.collective_compute(
                            "AllGather",
                            mybir.AluOpType.bypass,
                            replica_groups=[list(range(cfg.nc))],
                            ins=[nxq[l][qq][:]],
                            outs=[nxf[l][qq][:]],
                        )

    nc.compile()
    return nc


# ---------------------------------------------------------------------------
# Entry point
# ---------------------------------------------------------------------------

def _prep_inputs(cfg: Cfg, inputs: dict):
    """Build per-core in_maps + the permutation info for unsharding."""
    import ml_dtypes

    x = np.asarray(inputs["x"], dtype=np.float32)
    edge_index = np.asarray(inputs["edge_index"])

    sched, idx_bufs, dstloc_bufs, cnt_bufs = build_schedule_and_data(cfg, edge_index)

    # shard-layout packed x (for per-core xs slices + output unshard map)
    new_ids = _relabel(cfg, np.arange(cfg.n_real, dtype=np.int64))
    xpad = np.zeros((cfg.npad, cfg.d), dtype=np.float32)
    xpad[new_ids] = x
    xpacked = pack_hi_lo(xpad)
    # stage-major packed x (the gather table layout)
    all_sh = np.arange(cfg.npad, dtype=np.int64)
    pos = _pos_of_shard_id(cfg, sched, all_sh)
    xpacked_pos = np.empty_like(xpacked)
    xpacked_pos[pos] = xpacked

    iota = np.tile(np.arange(P, dtype=np.float32), (P, 1)).astype(ml_dtypes.bfloat16)
    ident = np.eye(P, dtype=np.float32)

    common = {
        "xp": xpacked_pos,
        "iota": np.ascontiguousarray(iota),
        "id16": np.ascontiguousarray(ident.astype(ml_dtypes.bfloat16)),
        "idf": ident,
        "woutb": np.tile(
            np.asarray(inputs["Wout"], np.float32).reshape(1, cfg.d), (P, 1)
        ),
        "boutb": np.full((P, 1), np.float32(np.asarray(inputs["bout"]).reshape(-1)[0])),
    }
    for l in range(3):
        common[f"wr{l}"] = np.asarray(inputs[f"Wr{l}"], np.float32)
        common[f"wn{l}"] = np.asarray(inputs[f"Wn{l}"], np.float32)
        common[f"bias{l}"] = np.tile(
            np.asarray(inputs[f"b{l}"], np.float32).reshape(1, cfg.d), (P, 1)
        )

    in_maps = []
    for c in range(cfg.nc):
        m = dict(common)
        m["xs"] = np.ascontiguousarray(xpacked[c * cfg.shard:(c + 1) * cfg.shard])
        m["idx"] = idx_bufs[c]
        m["dstloc"] = dstloc_bufs[c].astype(ml_dtypes.bfloat16)
        m["gcnt"] = cnt_bufs[c].reshape(1, -1)
        in_maps.append(m)

    return sched, in_maps, new_ids


def kernel(**inputs) -> np.ndarray:
    cfg = FULL
    sched, in_maps, new_ids = _prep_inputs(cfg, inputs)
    nc = build_program(cfg, sched)

    from concourse.bass_utils import run_bass_kernel_spmd

    res = run_bass_kernel_spmd(nc, in_maps, core_ids=list(range(cfg.nc)))
    full = np.concatenate([res.results[c]["out"] for c in range(cfg.nc)], axis=0)
    return np.ascontiguousarray(full[new_ids]).astype(np.float32)

